# revision 20
# baseline (speedup 1.0000x reference)
"""Trainium2 SPMD kernel for nn_Block_28157805592870 (dense transformer block).

Strategy (8 NeuronCores, tensor-parallel):
  - Heads sharded 2-per-core for attention; LN1 folded into the qkv weights
    host-side (diag(ln_w) fold + row-centering removes the mean subtraction;
    the per-token rsqrt(var) is precomputed on host and applied on device).
  - Softmax without max-subtraction (scores are O(8), exp is fp32-safe) and
    with deferred normalization (o = (exp @ v_aug) / sum via a ones column).
  - fp32 matmuls run in PE float32r (replication) mode: full fp32 data at
    1 cycle/row for moving dims >= 256 (plain fp32 is 4 cycles/row).
  - Per-head attention outputs move hd-sharded -> token-sharded with one small
    AllToAll (2.1 MB/core); proj + LN2 + MLP then run token-sharded (512
    tokens/core) with full weights, MLP matmuls in bf16.
  - The [H,T,T] mean attention map is produced by a second, natural-orientation
    score pass whose exp fuses the 1/(2*sum_b) factor via a -log bias; the two
    batches are averaged on DVE into 2048-wide row slabs and streamed out
    causally (upper triangle relies on the pre-zeroed PJRT output buffers).

kernel(**inputs) takes the full unsharded inputs and returns
(x_out [B,T,C], attn_mean [H,T,T]) like reference.reference().
"""

import numpy as np
import ml_dtypes

import jax
import jax.numpy as jnp
from jax.sharding import Mesh, PartitionSpec, NamedSharding
from jax.experimental.shard_map import shard_map as _shard_map

import concourse.bacc as bacc
import concourse.mybir as mybir
from concourse import bass, tile, bass2jax
from concourse.masks import make_identity

F32 = mybir.dt.float32
F32R = mybir.dt.float32r
BF16 = mybir.dt.bfloat16
AF = mybir.ActivationFunctionType
OP = mybir.AluOpType

B, T, C, H = 2, 2048, 1024, 16
D = C // H            # 64
NC = 8
BT = B * T            # 4096 tokens
TPC = BT // NC        # 512 tokens per core
EPS = 1e-5
SCALE = 1.0 / np.sqrt(D)  # 0.125


def _declare(nc):
    xt = nc.dram_tensor("xt", [C, BT], F32R, kind="ExternalInput").ap()
    xsl = nc.dram_tensor("xsl", [TPC, C], F32, kind="ExternalInput").ap()
    wq = nc.dram_tensor("wq", [C, 128], F32R, kind="ExternalInput").ap()
    wk = nc.dram_tensor("wk", [C, 128], F32R, kind="ExternalInput").ap()
    wv = nc.dram_tensor("wv", [C, 128], F32R, kind="ExternalInput").ap()
    bq = nc.dram_tensor("bq", [128, 1], F32, kind="ExternalInput").ap()
    bk = nc.dram_tensor("bk", [128, 1], F32, kind="ExternalInput").ap()
    bv = nc.dram_tensor("bv", [128, 1], F32, kind="ExternalInput").ap()
    rpt = nc.dram_tensor("rpt", [128, 32], F32, kind="ExternalInput").ap()
    rrow = nc.dram_tensor("rrow", [1, BT], F32, kind="ExternalInput").ap()
    wp = nc.dram_tensor("wp", [C, C], BF16, kind="ExternalInput").ap()
    bp = nc.dram_tensor("bp", [1, C], F32, kind="ExternalInput").ap()
    wf = nc.dram_tensor("wf", [C, 4 * C], BF16, kind="ExternalInput").ap()
    bf = nc.dram_tensor("bf", [128, 32], F32, kind="ExternalInput").ap()
    wf2 = nc.dram_tensor("wf2", [4 * C, C], BF16, kind="ExternalInput").ap()

    oy = nc.dram_tensor("oy", [TPC, C], F32, kind="ExternalOutput").ap()
    oa = nc.dram_tensor("oa", [2, T, T], F32, kind="ExternalOutput").ap()
    return (xt, xsl, wq, wk, wv, bq, bk, bv, rpt, rrow, wp, bp, wf, bf, wf2, oy, oa)


def _build_program():
    nc = bacc.Bacc("TRN2", target_bir_lowering=False, debug=False, num_devices=NC)
    ios = _declare(nc)
    with tile.TileContext(nc) as tc:
        _emit(nc, tc, *ios)
    nc.compile()
    return nc


def _emit(nc, tc, xt, xsl, wq, wk, wv, bq, bk, bv, rpt, rrow,
          wp, bp, wf, bf, wf2, oy, oa):
    mm = nc.tensor.matmul
    act = nc.scalar.activation

    with (
        tc.tile_pool(name="pw", bufs=6, space="PSUM") as pw,       # transient psum
        tc.tile_pool(name="pa", bufs=2, space="PSUM") as pa,       # accum psum
        tc.tile_pool(name="dram", bufs=1, space="DRAM") as dram,
        tc.tile_pool(name="consts", bufs=1) as consts,
        tc.tile_pool(name="persist", bufs=1) as persist,
    ):
        a2a_in = dram.tile([NC, 128, TPC], BF16)
        a2a_out = dram.tile([NC, 128, TPC], BF16)
        sums_d = dram.tile([2, B, 32, 128], F32)   # (head, batch, j, p): t = 128j+p

        ident = consts.tile([128, 128], F32)
        make_identity(nc, ident[:, :])
        ident_r = consts.tile([128, 128], F32R)
        nc.vector.tensor_copy(ident_r[:, :], ident[:, :])
        eps_col = consts.tile([128, 1], F32)
        nc.gpsimd.memset(eps_col[:, :], EPS)
        ones_col = consts.tile([128, 1], F32)
        nc.gpsimd.memset(ones_col[:, :], 1.0)

        wq_sb = consts.tile([128, 8, 128], F32R)
        wk_sb = consts.tile([128, 8, 128], F32R)
        wv_sb = consts.tile([128, 8, 128], F32R)
        nc.sync.dma_start(wq_sb[:, :, :], wq.rearrange("(a p) n -> p a n", p=128))
        nc.sync.dma_start(wk_sb[:, :, :], wk.rearrange("(a p) n -> p a n", p=128))
        nc.sync.dma_start(wv_sb[:, :, :], wv.rearrange("(a p) n -> p a n", p=128))
        bq_sb = consts.tile([128, 1], F32)
        bk_sb = consts.tile([128, 1], F32)
        bv_col = consts.tile([128, 1], F32)
        bp_sb = consts.tile([1, C], F32)
        bf_sb = consts.tile([128, 32], F32)
        nc.sync.dma_start(bq_sb[:, :], bq[:, :])
        nc.sync.dma_start(bk_sb[:, :], bk[:, :])
        nc.sync.dma_start(bv_col[:, :], bv[:, :])
        nc.sync.dma_start(bp_sb[:, :], bp[:, :])
        nc.sync.dma_start(bf_sb[:, :], bf[:, :])
        bp_bc = consts.tile([128, C], F32)
        nc.gpsimd.partition_broadcast(bp_bc[:, :], bp_sb[:, :])

        # persistent attention operands
        qT = persist.tile([128, BT], F32R)          # rows: h0 d0..63 | h1 d0..63
        kT = persist.tile([128, BT], F32R)

        # pools released after pass A: v_aug + LN1 r + phase-0 streaming
        attn_scope = tc.tile_pool(name="attn_scope", bufs=1)
        vstat = attn_scope.__enter__()
        v_aug = vstat.tile([128, 32, 130], F32R)    # (p, tokchunk, [d0..63,1,d0..63,1])
        r_pt = vstat.tile([128, 32], F32)
        r_row = vstat.tile([1, BT], F32)
        r_bc = vstat.tile([128, BT], F32)
        nc.sync.dma_start(r_pt[:, :], rpt[:, :])
        nc.sync.dma_start(r_row[:, :], rrow[:, :])
        nc.gpsimd.partition_broadcast(r_bc[:, :], r_row[:, :])

        # ---------------- phase 0: stream x^T -> q^T,k^T,v^T (r folded at evict) ----------------
        xt3 = xt.rearrange("(a p) t -> p a t", p=128)
        with (
            tc.tile_pool(name="xtp", bufs=3) as xtp,
            tc.tile_pool(name="vtp", bufs=2) as vtp,
        ):
            nc.vector.tensor_copy(v_aug[:, :, 64:65], ones_col[:, :].to_broadcast((128, 32, 1)))
            nc.vector.tensor_copy(v_aug[:, :, 129:130], ones_col[:, :].to_broadcast((128, 32, 1)))
            for tt in range(8):
                t0 = tt * 512
                xt_t = xtp.tile([128, 8, 512], F32R, tag="xt")
                nc.sync.dma_start(xt_t[:, :, :], xt3[:, :, t0:t0 + 512])

                ps_q = pw.tile([128, 512], F32, tag="w")
                ps_k = pw.tile([128, 512], F32, tag="w")
                ps_vt = pw.tile([128, 512], F32, tag="w")
                for ci in range(8):
                    st = ci == 0
                    sp = ci == 7
                    mm(ps_q[:, :], (wq_sb[:, ci, :]), (xt_t[:, ci, :]), start=st, stop=sp)
                    mm(ps_k[:, :], (wk_sb[:, ci, :]), (xt_t[:, ci, :]), start=st, stop=sp)
                    mm(ps_vt[:, :], (wv_sb[:, ci, :]), (xt_t[:, ci, :]), start=st, stop=sp)
                # evict with the per-token LN1 scale folded in; bias per-partition after
                nc.vector.tensor_tensor(qT[:, t0:t0 + 512], ps_q[:, :],
                                        r_bc[:, t0:t0 + 512], OP.mult)
                nc.vector.tensor_scalar_add(qT[:, t0:t0 + 512], qT[:, t0:t0 + 512], bq_sb[:, :])
                nc.vector.tensor_tensor(kT[:, t0:t0 + 512], ps_k[:, :],
                                        r_bc[:, t0:t0 + 512], OP.mult)
                nc.vector.tensor_scalar_add(kT[:, t0:t0 + 512], kT[:, t0:t0 + 512], bk_sb[:, :])
                vt_t = vtp.tile([128, 512], F32R, tag="vt")
                nc.vector.tensor_tensor(vt_t[:, :], ps_vt[:, :],
                                        r_bc[:, t0:t0 + 512], OP.mult)
                nc.vector.tensor_scalar_add(vt_t[:, :], vt_t[:, :], bv_col[:, :])
                # transpose v^T -> v chunks of v_aug
                for sc in range(4):
                    ps_tr = pw.tile([128, 128], F32R, tag="w", name="ps_tr")
                    nc.tensor.transpose(ps_tr[:, :], vt_t[:, sc * 128:(sc + 1) * 128],
                                        ident_r[:, :])
                    chunk = tt * 4 + sc
                    nc.vector.tensor_copy(v_aug[:, chunk, 0:64], ps_tr[:, 0:64])
                    nc.vector.tensor_copy(v_aug[:, chunk, 65:129], ps_tr[:, 64:128])

        # ---------------- pass A: scores^T -> exp -> o^T (per head/batch) ----------------
        with (
            tc.tile_pool(name="ptp", bufs=4) as ptp,
            tc.tile_pool(name="osb", bufs=4) as osb,
        ):
            for b in range(B):
                for qi in range(4):
                    po = {}
                    for h in range(2):
                        po[h] = pa.tile([65, 512], F32, tag="a", name=f"po{h}")
                    nkc = 4 * (qi + 1)
                    for j in range(nkc):
                        qlo = max(0, 128 * (j - 4 * qi))
                        for h in range(2):
                            hp = 64 * h
                            ps = pw.tile([128, 512], F32, tag="w")
                            mm(ps[:, qlo:512],
                               (kT[hp:hp + 64, 2048 * b + 128 * j: 2048 * b + 128 * (j + 1)]),
                               (qT[hp:hp + 64, 2048 * b + 512 * qi + qlo: 2048 * b + 512 * (qi + 1)]),
                               start=True, stop=True)
                            pT = ptp.tile([128, 512], F32R, tag="pt")
                            act(pT[:, qlo:512], ps[:, qlo:512], AF.Exp, scale=SCALE)
                            if j >= 4 * qi:
                                # zero k>q within the diagonal 128x128 block
                                nc.gpsimd.affine_select(
                                    out=pT[:, qlo:qlo + 128], in_=pT[:, qlo:qlo + 128],
                                    compare_op=OP.is_ge, fill=0.0, base=0,
                                    pattern=[[1, 128]], channel_multiplier=-1)
                            mm(po[h][:, qlo:512],
                               (v_aug[:, b * 16 + j, 65 * h: 65 * h + 65]),
                               (pT[:, qlo:512]),
                               start=(j == 0), stop=(j == nkc - 1))
                    for h in range(2):
                        oT = osb.tile([65, 512], F32, tag="ot")
                        nc.vector.tensor_copy(oT[:, :], po[h][:, :])
                        inv = osb.tile([1, 512], F32, tag="inv")
                        nc.vector.reciprocal(inv[:, :], oT[64:65, :])
                        ibc = osb.tile([64, 512], F32, tag="ibc")
                        nc.gpsimd.partition_broadcast(ibc[:, :], inv[:, :])
                        on = osb.tile([64, 512], BF16, tag="on")
                        nc.vector.tensor_tensor(on[:, :], oT[0:64, :], ibc[:, :], OP.mult)
                        nc.sync.dma_start(a2a_in[b * 4 + qi, 64 * h:64 * (h + 1), :], on[:, :])
                        nc.sync.dma_start(sums_d[h, b, qi * 4:(qi + 1) * 4, :], oT[64:65, :])

        attn_scope.__exit__(None, None, None)

        nc.gpsimd.collective_compute(
            "AllToAll", OP.bypass, replica_groups=[list(range(NC))],
            ins=[a2a_in.opt()], outs=[a2a_out.opt()])

        # ---------------- pass B: natural scores -> mean attention map ----------------
        # (kept in the same pool scope as the MLP so the scheduler can overlap them)
        with (
            tc.tile_pool(name="nlp", bufs=1) as nlp,
            tc.tile_pool(name="mbp", bufs=6) as mbp,
            tc.tile_pool(name="arp", bufs=1) as arp,
            tc.tile_pool(name="xmp", bufs=1) as xmp,
            tc.tile_pool(name="lt2", bufs=1) as lt2,
            tc.tile_pool(name="mst", bufs=4) as mst,
            tc.tile_pool(name="wfq", bufs=1) as wfqp,
            tc.tile_pool(name="gwp", bufs=1) as gwp,
            tc.tile_pool(name="yp", bufs=2) as yp,
        ):
            nl = {}
            for h in range(2):
                for b in range(B):
                    s_pt = nlp.tile([128, 32], F32, tag=f"s{h}{b}", name=f"s{h}{b}")
                    nc.sync.dma_start(s_pt[:, :], sums_d[h, b].rearrange("j p -> p j"))
                    nlt = nlp.tile([128, 32], F32, tag=f"n{h}{b}", name=f"n{h}{b}")
                    act(nlt[:, :], s_pt[:, :], AF.Ln, scale=2.0)
                    nc.vector.tensor_scalar_mul(nlt[:, :], nlt[:, :], -1.0)
                    nl[(h, b)] = nlt

            for h in range(2):
                for a in range(16):
                    width = 128 * a + 128
                    arow = arp.tile([128, 2048], F32, tag="ar")
                    for j in range(a // 4 + 1):
                        N = 512 if j < a // 4 else 128 * (a % 4) + 128
                        mts = []
                        for b in range(B):
                            hp = 64 * h
                            ps = pa.tile([128, 512], F32, tag="a", name="psb")
                            mm(ps[:, :N],
                               (qT[hp:hp + 64, 2048 * b + 128 * a: 2048 * b + 128 * (a + 1)]),
                               (kT[hp:hp + 64, 2048 * b + 512 * j: 2048 * b + 512 * j + N]),
                               start=True, stop=True)
                            mt = mbp.tile([128, 512], F32, tag="mb")
                            act(mt[:, :N], ps[:, :N], AF.Exp, scale=SCALE,
                                bias=nl[(h, b)][:, a:a + 1])
                            mts.append(mt)
                        if j == a // 4:
                            for mt in mts:
                                nc.gpsimd.affine_select(
                                    out=mt[:, N - 128:N], in_=mt[:, N - 128:N],
                                    compare_op=OP.is_ge, fill=0.0, base=0,
                                    pattern=[[-1, 128]], channel_multiplier=1)
                        nc.vector.tensor_tensor(arow[:, 512 * j:512 * j + N],
                                                mts[0][:, :N], mts[1][:, :N], OP.add)
                    nc.sync.dma_start(oa[h, 128 * a:128 * (a + 1), 0:width],
                                      arow[:, 0:width])

            # ---------------- token-sharded proj + residual ----------------
            xm_sb = xmp.tile([128, 4, C], F32)
            r2_col = xmp.tile([128, 4], F32)
            ln2T = lt2.tile([128, 8, TPC], BF16)
            with (
                tc.tile_pool(name="ocp", bufs=1) as ocp,
                tc.tile_pool(name="wpp", bufs=2) as wpp,
                tc.tile_pool(name="ln2p", bufs=2) as ln2p,
            ):
                oc_sb = ocp.tile([128, 8, TPC], BF16)
                nc.sync.dma_start(oc_sb[:, :, :], a2a_out.rearrange("a p t -> p a t"))
                for t4 in range(4):
                    ps0 = pw.tile([128, 512], F32, tag="w")
                    ps1 = pw.tile([128, 512], F32, tag="w")
                    for ci in range(8):
                        wp_t = wpp.tile([128, C], BF16, tag="wp")
                        nc.sync.dma_start(wp_t[:, :], wp[ci * 128:(ci + 1) * 128, :])
                        lh = oc_sb[:, ci, 128 * t4:128 * (t4 + 1)]
                        mm(ps0[:, :], (lh), (wp_t[:, 0:512]), start=(ci == 0), stop=(ci == 7))
                        mm(ps1[:, :], (lh), (wp_t[:, 512:C]), start=(ci == 0), stop=(ci == 7))
                    xsl_t = ln2p.tile([128, C], F32, tag="xs")
                    nc.sync.dma_start(xsl_t[:, :], xsl[128 * t4:128 * (t4 + 1), :])
                    nc.vector.tensor_tensor(xm_sb[:, t4, 0:512], ps0[:, :], xsl_t[:, 0:512], OP.add)
                    nc.vector.tensor_tensor(xm_sb[:, t4, 512:C], ps1[:, :], xsl_t[:, 512:C], OP.add)
                    nc.vector.tensor_tensor(xm_sb[:, t4, :], xm_sb[:, t4, :], bp_bc[:, :], OP.add)

                    # ---- LN2 + transpose to ln2T (bf16) ----
                    bst = mst.tile([128, 2, 6], F32, tag="bst")
                    nc.vector.bn_stats(bst[:, 0, :], xm_sb[:, t4, 0:512])
                    nc.vector.bn_stats(bst[:, 1, :], xm_sb[:, t4, 512:C])
                    mv = mst.tile([128, 2], F32, tag="mv")
                    nc.vector.bn_aggr(mv[:, :], bst[:, :, :])
                    std2 = mst.tile([128, 1], F32, tag="sd2")
                    act(std2[:, :], mv[:, 1:2], AF.Sqrt, bias=eps_col[:, :])
                    nc.vector.reciprocal(r2_col[:, t4:t4 + 1], std2[:, :])
                    ln2_t = ln2p.tile([128, C], F32, tag="ln2")
                    nc.vector.scalar_tensor_tensor(
                        ln2_t[:, :], xm_sb[:, t4, :], mv[:, 0:1],
                        r2_col[:, t4:t4 + 1].to_broadcast((128, C)),
                        OP.subtract, OP.mult)
                    for ci in range(8):
                        pst = pw.tile([128, 128], F32, tag="w")
                        nc.tensor.transpose(pst[:, :], ln2_t[:, ci * 128:(ci + 1) * 128], ident[:, :])
                        nc.vector.tensor_copy(ln2T[:, ci, 128 * t4:128 * (t4 + 1)], pst[:, :])

            # ---------------- MLP in quarters of the hidden dim ----------------
            wf3 = wf.rearrange("(a p) n -> p a n", p=128)
            mlp_part = xmp.tile([128, 4, C], F32)
            NQ = 4
            MQ = 32 // NQ
            for quar in range(NQ):
                wfq_t = wfqp.tile([128, 8, MQ * 128], BF16, tag="wfq", name="wfq_t")
                nc.sync.dma_start(wfq_t[:, :, :],
                                  wf3[:, :, quar * MQ * 128:(quar + 1) * MQ * 128])
                g_q = gwp.tile([128, MQ, TPC], BF16, tag="g", name="g_q")
                wf2_q = gwp.tile([128, MQ, C], BF16, tag="wf2", name="wf2_q")
                for m in range(MQ):
                    mg = quar * MQ + m
                    ps1 = pw.tile([128, 512], F32, tag="w", name="psf1")
                    for ci in range(8):
                        mm(ps1[:, :], wfq_t[:, ci, m * 128:(m + 1) * 128], ln2T[:, ci, :],
                           start=(ci == 0), stop=(ci == 7))
                    act(g_q[:, m, :], ps1[:, :], AF.Gelu, bias=bf_sb[:, mg:mg + 1])
                    nc.sync.dma_start(wf2_q[:, m, :], wf2[mg * 128:(mg + 1) * 128, :])
                for t4 in range(4):
                    ps0 = pa.tile([128, 512], F32, tag="a", name="psf2a")
                    ps1 = pa.tile([128, 512], F32, tag="a", name="psf2b")
                    for m in range(MQ):
                        lh = g_q[:, m, 128 * t4:128 * (t4 + 1)]
                        mm(ps0[:, :], lh, wf2_q[:, m, 0:512], start=(m == 0), stop=(m == MQ - 1))
                        mm(ps1[:, :], lh, wf2_q[:, m, 512:C], start=(m == 0), stop=(m == MQ - 1))
                    if quar == 0:
                        nc.vector.tensor_copy(mlp_part[:, t4, 0:512], ps0[:, :])
                        nc.vector.tensor_copy(mlp_part[:, t4, 512:C], ps1[:, :])
                    elif quar < NQ - 1:
                        nc.vector.tensor_tensor(mlp_part[:, t4, 0:512], ps0[:, :],
                                                mlp_part[:, t4, 0:512], OP.add)
                        nc.vector.tensor_tensor(mlp_part[:, t4, 512:C], ps1[:, :],
                                                mlp_part[:, t4, 512:C], OP.add)
                    else:
                        y_t = yp.tile([128, C], F32, tag="y", name="y_t")
                        nc.vector.tensor_tensor(y_t[:, 0:512], ps0[:, :],
                                                mlp_part[:, t4, 0:512], OP.add)
                        nc.vector.tensor_tensor(y_t[:, 512:C], ps1[:, :],
                                                mlp_part[:, t4, 512:C], OP.add)
                        nc.vector.tensor_tensor(y_t[:, :], y_t[:, :], xm_sb[:, t4, :], OP.add)
                        nc.sync.dma_start(oy[128 * t4:128 * (t4 + 1), :], y_t[:, :])


# ----------------------------------------------------------------------------
# Host side: runner with cached compile, input prep, output assembly
# ----------------------------------------------------------------------------

_STATE = {}


def _get_runner():
    if "run" in _STATE:
        return _STATE["run"]

    nc = _build_program()
    bass2jax.install_neuronx_cc_hook()

    partition_name = nc.partition_id_tensor.name if nc.partition_id_tensor else None
    in_names, out_names, out_avals = [], [], []
    for alloc in nc.m.functions[0].allocations:
        if not isinstance(alloc, mybir.MemoryLocationSet):
            continue
        name = alloc.memorylocations[0].name
        if alloc.kind == "ExternalInput":
            if name != partition_name:
                in_names.append(name)
        elif alloc.kind == "ExternalOutput":
            shape = tuple(alloc.tensor_shape)
            dtype = mybir.dt.np(alloc.dtype)
            out_names.append(name)
            out_avals.append(jax.core.ShapedArray(shape, dtype))
    n_params = len(in_names)
    full_in_names = in_names + out_names
    if partition_name is not None:
        full_in_names.append(partition_name)

    donate = tuple(range(n_params, n_params + len(out_names)))

    def _body(*args):
        operands = list(args)
        if partition_name is not None:
            operands.append(bass2jax.partition_id_tensor())
        outs = bass2jax._bass_exec_p.bind(
            *operands,
            out_avals=tuple(out_avals),
            in_names=tuple(full_in_names),
            out_names=tuple(out_names),
            lowering_input_output_aliases=(),
            sim_require_finite=True,
            sim_require_nnan=True,
            nc=nc,
        )
        return tuple(outs)

    devices = jax.devices()[:NC]
    mesh = Mesh(np.asarray(devices), ("core",))
    n_in = n_params + len(out_names)
    sharded = jax.jit(
        _shard_map(_body, mesh=mesh,
                   in_specs=(PartitionSpec("core"),) * n_in,
                   out_specs=(PartitionSpec("core"),) * len(out_names),
                   check_rep=False),
        donate_argnums=donate, keep_unused=True)

    sharding = NamedSharding(mesh, PartitionSpec("core"))
    zero_shapes = [(NC * av.shape[0], *av.shape[1:]) for av in out_avals]
    zero_dtypes = [av.dtype for av in out_avals]

    @jax.jit
    def _mk_zeros():
        return tuple(jnp.zeros(s, d) for s, d in zip(zero_shapes, zero_dtypes))

    def make_zeros():
        return jax.device_put(_mk_zeros(), tuple([sharding] * len(out_avals)))

    def put_inputs(in_maps):
        arrs = []
        for name in in_names:
            cat = np.concatenate([np.asarray(m[name]) for m in in_maps], axis=0)
            arrs.append(jax.device_put(cat, sharding))
        return arrs

    runner = {
        "nc": nc, "sharded": sharded, "in_names": in_names, "out_names": out_names,
        "out_avals": out_avals, "make_zeros": make_zeros, "put_inputs": put_inputs,
        "mesh": mesh,
    }
    _STATE["run"] = runner
    return runner


def _round_f32r(a):
    """Round fp32 to the PE's fp32r format (11-bit mantissa, RNE-ish)."""
    b = np.ascontiguousarray(a, np.float32).view(np.uint32)
    b = ((b + np.uint32(0x800)) & np.uint32(0xFFFFF000)).astype(np.uint32)
    return b.view(np.float32)


def _prep_inputs(inputs):
    x = np.asarray(inputs["x"], np.float32)
    ln1_w = np.asarray(inputs["ln1_w"], np.float32)
    ln1_b = np.asarray(inputs["ln1_b"], np.float32)
    w_qkv = np.asarray(inputs["w_qkv"], np.float32)
    b_qkv = np.asarray(inputs["b_qkv"], np.float32)
    w_proj = np.asarray(inputs["w_proj"], np.float32)
    b_proj = np.asarray(inputs["b_proj"], np.float32)
    ln2_w = np.asarray(inputs["ln2_w"], np.float32)
    ln2_b = np.asarray(inputs["ln2_b"], np.float32)
    w_fc = np.asarray(inputs["w_fc"], np.float32)
    b_fc = np.asarray(inputs["b_fc"], np.float32)
    w_fc2 = np.asarray(inputs["w_fc2"], np.float32)
    b_fc2 = np.asarray(inputs["b_fc2"], np.float32)

    x2d = np.ascontiguousarray(x.reshape(BT, C))
    xt_r = _round_f32r(np.ascontiguousarray(x2d.T))

    # LN1 per-token inverse std (cheap O(BT*C) host work)
    s = x2d.sum(axis=1, dtype=np.float32)
    ss = np.einsum("tc,tc->t", x2d, x2d, dtype=np.float32)
    var = ss / C - (s / C) ** 2
    r = (1.0 / np.sqrt(var + EPS)).astype(np.float32)

    Wd = ln1_w[:, None] * w_qkv
    Wc = np.ascontiguousarray(Wd - Wd.mean(axis=0, keepdims=True))
    bias_qkv = ln1_b @ w_qkv + b_qkv

    Wdfc = (ln2_w[:, None] * w_fc).astype(ml_dtypes.bfloat16)
    bias_fc = (ln2_b @ w_fc + b_fc).astype(np.float32)
    wf2_bf = w_fc2.astype(ml_dtypes.bfloat16)

    in_maps = []
    for c in range(NC):
        qcols = slice(128 * c, 128 * (c + 1))
        kcols = slice(C + 128 * c, C + 128 * (c + 1))
        vcols = slice(2 * C + 128 * c, 2 * C + 128 * (c + 1))
        in_maps.append({
            "xt": xt_r,
            "xsl": np.ascontiguousarray(x2d[TPC * c:TPC * (c + 1)]),
            "wq": _round_f32r(Wc[:, qcols]),
            "wk": _round_f32r(Wc[:, kcols]),
            "wv": _round_f32r(Wc[:, vcols]),
            "bq": np.ascontiguousarray(bias_qkv[qcols].reshape(128, 1)),
            "bk": np.ascontiguousarray(bias_qkv[kcols].reshape(128, 1)),
            "bv": np.ascontiguousarray(bias_qkv[vcols].reshape(128, 1)),
            "rpt": np.ascontiguousarray(r.reshape(32, 128).T),
            "rrow": np.ascontiguousarray(r.reshape(1, BT)),
            "wp": w_proj.astype(ml_dtypes.bfloat16),
            "bp": np.ascontiguousarray(b_proj.reshape(1, C)),
            "wf": Wdfc,
            "bf": np.ascontiguousarray(bias_fc.reshape(32, 128).T),
            "wf2": wf2_bf,
        })
    return in_maps, b_fc2


def _assemble(per_core, b_fc2):
    x_out = np.concatenate([per_core[c]["oy"] for c in range(NC)], axis=0)
    x_out = (x_out + b_fc2[None, :]).astype(np.float32).reshape(B, T, C)
    attn = np.concatenate([per_core[c]["oa"] for c in range(NC)], axis=0)
    return x_out, attn


def run_on_device(in_maps):
    """Execute one step; returns per-core dict of outputs (host numpy)."""
    r = _get_runner()
    in_dev = r["put_inputs"](in_maps)
    zeros = r["make_zeros"]()
    outs = r["sharded"](*in_dev, *zeros)
    per_core = []
    for c in range(NC):
        d = {}
        for i, name in enumerate(r["out_names"]):
            av = r["out_avals"][i]
            d[name] = np.asarray(outs[i]).reshape(NC, *av.shape)[c]
        per_core.append(d)
    return per_core


def kernel(**inputs):
    in_maps, b_fc2 = _prep_inputs(inputs)
    per_core = run_on_device(in_maps)
    return _assemble(per_core, b_fc2)


if __name__ == "__main__":
    nc = _build_program()
    print("build ok:", len(nc.m.functions[0].allocations), "allocations")


# revision 32
# speedup vs baseline: 15.3400x; 15.3400x over previous
"""Trainium2 SPMD kernel for nn_Block_28157805592870 (dense transformer block).

Strategy (8 NeuronCores, tensor-parallel):
  - Heads sharded 2-per-core for attention; LN1 folded into the qkv weights
    host-side (diag(ln_w) fold + row-centering removes the mean subtraction;
    the per-token rsqrt(var) is precomputed on host and applied on device).
  - Softmax without max-subtraction (scores are O(8), exp is fp32-safe) and
    with deferred normalization (o = (exp @ v_aug) / sum via a ones column).
  - fp32 matmuls run in PE float32r (replication) mode: full fp32 data at
    1 cycle/row for moving dims >= 256 (plain fp32 is 4 cycles/row).
  - Per-head attention outputs move hd-sharded -> token-sharded with one small
    AllToAll (2.1 MB/core); proj + LN2 + MLP then run token-sharded (512
    tokens/core) with full weights, MLP matmuls in bf16.
  - The [H,T,T] mean attention map is produced by a second, natural-orientation
    score pass whose exp fuses the 1/(2*sum_b) factor via a -log bias; the two
    batches are averaged on DVE into 2048-wide row slabs and streamed out
    causally (upper triangle relies on the pre-zeroed PJRT output buffers).

kernel(**inputs) takes the full unsharded inputs and returns
(x_out [B,T,C], attn_mean [H,T,T]) like reference.reference().
"""

import numpy as np
import ml_dtypes

import jax
import jax.numpy as jnp
from jax.sharding import Mesh, PartitionSpec, NamedSharding
from jax.experimental.shard_map import shard_map as _shard_map

import concourse.bacc as bacc
import concourse.mybir as mybir
from concourse import bass, tile, bass2jax
from concourse.masks import make_identity

F32 = mybir.dt.float32
F32R = mybir.dt.float32r
BF16 = mybir.dt.bfloat16
AF = mybir.ActivationFunctionType
OP = mybir.AluOpType

B, T, C, H = 2, 2048, 1024, 16
D = C // H            # 64
NC = 8
BT = B * T            # 4096 tokens
TPC = BT // NC        # 512 tokens per core
EPS = 1e-5
SCALE = 1.0 / np.sqrt(D)  # 0.125


def _declare(nc):
    xt = nc.dram_tensor("xt", [C, BT], F32R, kind="ExternalInput").ap()
    xsl = nc.dram_tensor("xsl", [TPC, C], F32, kind="ExternalInput").ap()
    wq = nc.dram_tensor("wq", [C, 128], F32R, kind="ExternalInput").ap()
    wk = nc.dram_tensor("wk", [C, 128], F32R, kind="ExternalInput").ap()
    wv = nc.dram_tensor("wv", [C, 128], F32R, kind="ExternalInput").ap()
    bq = nc.dram_tensor("bq", [128, 1], F32, kind="ExternalInput").ap()
    bk = nc.dram_tensor("bk", [128, 1], F32, kind="ExternalInput").ap()
    bv = nc.dram_tensor("bv", [128, 1], F32, kind="ExternalInput").ap()
    wp = nc.dram_tensor("wp", [C, C], BF16, kind="ExternalInput").ap()
    bp = nc.dram_tensor("bp", [1, C], F32, kind="ExternalInput").ap()
    wf = nc.dram_tensor("wf", [C, 4 * C], BF16, kind="ExternalInput").ap()
    bf = nc.dram_tensor("bf", [128, 32], F32, kind="ExternalInput").ap()
    wf2 = nc.dram_tensor("wf2", [4 * C, C], BF16, kind="ExternalInput").ap()

    oy = nc.dram_tensor("oy", [TPC, C], F32, kind="ExternalOutput").ap()
    oa = nc.dram_tensor("oa", [2, T, T], F32, kind="ExternalOutput").ap()
    return (xt, xsl, wq, wk, wv, bq, bk, bv, wp, bp, wf, bf, wf2, oy, oa)


def _build_program():
    nc = bacc.Bacc("TRN2", target_bir_lowering=False, debug=False, num_devices=NC)
    ios = _declare(nc)
    with tile.TileContext(nc) as tc:
        _emit(nc, tc, *ios)
    nc.compile()
    return nc


def _emit(nc, tc, xt, xsl, wq, wk, wv, bq, bk, bv,
          wp, bp, wf, bf, wf2, oy, oa):
    mm = nc.tensor.matmul
    act = nc.scalar.activation

    with (
        tc.tile_pool(name="pw", bufs=2, space="PSUM") as pw,       # transient psum (2-bank slots)
        tc.tile_pool(name="pa", bufs=2, space="PSUM") as pa,       # accum psum
        tc.tile_pool(name="dram", bufs=1, space="DRAM") as dram,
        tc.tile_pool(name="consts", bufs=1) as consts,
        tc.tile_pool(name="persist", bufs=1) as persist,
    ):
        a2a_in = dram.tile([NC, 128, TPC], BF16)
        a2a_out = dram.tile([NC, 128, TPC], BF16)
        sums_d = dram.tile([2, B, 32, 128], F32)   # (head, batch, j, p): t = 128j+p

        ident = consts.tile([128, 128], F32)
        make_identity(nc, ident[:, :])
        ident_r = consts.tile([128, 128], F32R)
        nc.vector.tensor_copy(ident_r[:, :], ident[:, :])
        eps_col = consts.tile([128, 1], F32)
        nc.gpsimd.memset(eps_col[:, :], EPS)
        ones_col = consts.tile([128, 1], F32)
        nc.gpsimd.memset(ones_col[:, :], 1.0)

        wq_sb = consts.tile([128, 8, 128], F32R)
        wk_sb = consts.tile([128, 8, 128], F32R)
        wv_sb = consts.tile([128, 8, 128], F32R)
        nc.sync.dma_start(wq_sb[:, :, :], wq.rearrange("(a p) n -> p a n", p=128))
        nc.sync.dma_start(wk_sb[:, :, :], wk.rearrange("(a p) n -> p a n", p=128))
        nc.sync.dma_start(wv_sb[:, :, :], wv.rearrange("(a p) n -> p a n", p=128))
        bq_sb = consts.tile([128, 1], F32)
        bk_sb = consts.tile([128, 1], F32)
        bv_col = consts.tile([128, 1], F32)
        bp_sb = consts.tile([1, C], F32)
        bf_sb = consts.tile([128, 32], F32)
        nc.sync.dma_start(bq_sb[:, :], bq[:, :])
        nc.sync.dma_start(bk_sb[:, :], bk[:, :])
        nc.sync.dma_start(bv_col[:, :], bv[:, :])
        nc.sync.dma_start(bp_sb[:, :], bp[:, :])
        nc.sync.dma_start(bf_sb[:, :], bf[:, :])
        bp_bc = consts.tile([128, C], F32)
        nc.gpsimd.partition_broadcast(bp_bc[:, :], bp_sb[:, :])

        # persistent attention operands
        qT = persist.tile([128, BT], F32R)          # rows: h0 d0..63 | h1 d0..63
        kT = persist.tile([128, BT], F32R)

        nlp_cm = tc.tile_pool(name="nlp", bufs=1)
        mbp_cm = tc.tile_pool(name="mbp", bufs=4)
        arp_cm = tc.tile_pool(name="arp", bufs=2)
        ptp_cm = tc.tile_pool(name="ptp", bufs=4)
        osb_cm = tc.tile_pool(name="osb", bufs=4)
        nlp = nlp_cm.__enter__()
        mbp = mbp_cm.__enter__()
        arp = arp_cm.__enter__()
        ptp = ptp_cm.__enter__()
        osb = osb_cm.__enter__()
        # pools released after pass A: v_aug + LN1 r + phase-0 streaming
        attn_scope = tc.tile_pool(name="attn_scope", bufs=1)
        vstat = attn_scope.__enter__()
        v_aug = vstat.tile([128, 32, 130], F32R)    # (p, tokchunk, [d0..63,1,d0..63,1])

        # ---------------- phase 0: stream x^T -> q^T,k^T,v^T (r folded at evict) ----------------
        xt3 = xt.rearrange("(a p) t -> p a t", p=128)
        with (
            tc.tile_pool(name="xtp", bufs=3) as xtp,
            tc.tile_pool(name="vtp", bufs=2) as vtp,
        ):
            nc.vector.tensor_copy(v_aug[:, :, 64:65], ones_col[:, :].to_broadcast((128, 32, 1)))
            nc.vector.tensor_copy(v_aug[:, :, 129:130], ones_col[:, :].to_broadcast((128, 32, 1)))
            for tt in range(8):
                t0 = tt * 512
                xt_t = xtp.tile([128, 8, 512], F32R, tag="xt", name="xt_t")
                for ci in range(8):
                    nc.sync.dma_start(xt_t[:, ci, :], xt3[:, ci, t0:t0 + 512])

                ps_qk = pw.tile([128, 1024], F32, tag="w", name="ps_qk")
                ps_vt = pw.tile([128, 512], F32, tag="w", name="ps_vt")
                for ci in range(8):
                    st = ci == 0
                    sp = ci == 7
                    mm(ps_qk[:, 0:512], (wq_sb[:, ci, :]), (xt_t[:, ci, :]), start=st, stop=sp)
                    mm(ps_qk[:, 512:1024], (wk_sb[:, ci, :]), (xt_t[:, ci, :]), start=st, stop=sp)
                    mm(ps_vt[:, :], (wv_sb[:, ci, :]), (xt_t[:, ci, :]), start=st, stop=sp)
                # x^T arrives pre-scaled by the LN1 rsqrt (host); just add biases
                nc.vector.tensor_scalar(qT[:, t0:t0 + 512], ps_qk[:, 0:512],
                                        bq_sb[:, :], None, OP.add)
                nc.vector.tensor_scalar(kT[:, t0:t0 + 512], ps_qk[:, 512:1024],
                                        bk_sb[:, :], None, OP.add)
                vt_t = vtp.tile([128, 512], F32R, tag="vt", name="vt_t")
                nc.vector.tensor_scalar(vt_t[:, :], ps_vt[:, :],
                                        bv_col[:, :], None, OP.add)
                # transpose v^T -> v chunks of v_aug
                for sc in range(4):
                    ps_tr = pw.tile([128, 128], F32R, tag="w", name="ps_tr")
                    nc.tensor.transpose(ps_tr[:, :], vt_t[:, sc * 128:(sc + 1) * 128],
                                        ident_r[:, :])
                    chunk = tt * 4 + sc
                    nc.vector.tensor_copy(v_aug[:, chunk, 0:64], ps_tr[:, 0:64])
                    nc.vector.tensor_copy(v_aug[:, chunk, 65:129], ps_tr[:, 64:128])

        # ---------------- attention passes, per head, interleaved ----------------

        def passA_block(h, qi):
            """scores^T -> exp -> o^T for one (head, q-tile), both batches paired."""
            hp = 64 * h
            po = {}
            for b in range(B):
                po[b] = pa.tile([65, 512], F32, tag="a", name=f"po{b}")
            nkc = 4 * (qi + 1)
            for j in range(nkc):
                qlo = max(0, 128 * (j - 4 * qi))
                ps = pw.tile([128, 1024], F32, tag="w", name="ps_pair")
                pT = ptp.tile([128, 1024], F32R, tag="pt", name="pT")
                for b in range(B):
                    mm(ps[:, 512 * b + qlo: 512 * b + 512],
                       (kT[hp:hp + 64, 2048 * b + 128 * j: 2048 * b + 128 * (j + 1)]),
                       (qT[hp:hp + 64, 2048 * b + 512 * qi + qlo: 2048 * b + 512 * (qi + 1)]),
                       start=True, stop=True)
                psv = ps[:, :].rearrange("p (b q) -> p b q", b=2)[:, :, qlo:512]
                ptv = pT[:, :].rearrange("p (b q) -> p b q", b=2)[:, :, qlo:512]
                act(ptv, psv, AF.Exp, scale=SCALE)
                if j >= 4 * qi:
                    for b in range(B):
                        nc.gpsimd.affine_select(
                            out=pT[:, 512 * b + qlo: 512 * b + qlo + 128],
                            in_=pT[:, 512 * b + qlo: 512 * b + qlo + 128],
                            compare_op=OP.is_ge, fill=0.0, base=0,
                            pattern=[[1, 128]], channel_multiplier=-1)
                for b in range(B):
                    mm(po[b][:, qlo:512],
                       (v_aug[:, b * 16 + j, 65 * h: 65 * h + 65]),
                       (pT[:, 512 * b + qlo: 512 * b + 512]),
                       start=(j == 0), stop=(j == nkc - 1))
            for b in range(B):
                oT = osb.tile([65, 512], F32, tag="ot", name="oT")
                nc.vector.tensor_copy(oT[:, :], po[b][:, :])
                inv = osb.tile([1, 512], F32, tag="inv", name="inv")
                nc.vector.reciprocal(inv[:, :], oT[64:65, :])
                ibc = osb.tile([64, 512], F32, tag="ibc", name="ibc")
                nc.gpsimd.partition_broadcast(ibc[:, :], inv[:, :])
                on = osb.tile([64, 512], BF16, tag="on", name="on")
                nc.vector.tensor_tensor(on[:, :], oT[0:64, :], ibc[:, :], OP.mult)
                nc.sync.dma_start(a2a_in[b * 4 + qi, 64 * h:64 * (h + 1), :], on[:, :])
                nc.sync.dma_start(sums_d[h, b, qi * 4:(qi + 1) * 4, :], oT[64:65, :])

        sca = {}

        def prep_scales(h):
            """0.5 / sums, per batch, in [128, 32] token-chunk layout."""
            for b in range(B):
                s_pt = nlp.tile([128, 32], F32, tag=f"s{h}{b}", name=f"s{h}{b}")
                nc.sync.dma_start(s_pt[:, :], sums_d[h, b].rearrange("j p -> p j"))
                srec = nlp.tile([128, 32], F32, tag=f"r{h}{b}", name=f"r{h}{b}")
                nc.vector.reciprocal(srec[:, :], s_pt[:, :])
                nc.vector.tensor_scalar_mul(srec[:, :], srec[:, :], 0.5)
                sca[(h, b)] = srec

        def passB_chunk(h, a):
            """mean attention map rows [128a:128a+128] for head h."""
            hp = 64 * h
            width = 128 * a + 128
            arow = arp.tile([128, 2048], F32, tag="ar", name="arow")
            for j in range(a // 4 + 1):
                N = 512 if j < a // 4 else 128 * (a % 4) + 128
                ps = pa.tile([128, 1024], F32, tag="a", name="psb")
                for b in range(B):
                    mm(ps[:, 512 * b: 512 * b + N],
                       (qT[hp:hp + 64, 2048 * b + 128 * a: 2048 * b + 128 * (a + 1)]),
                       (kT[hp:hp + 64, 2048 * b + 512 * j: 2048 * b + 512 * j + N]),
                       start=True, stop=True)
                mt = mbp.tile([128, 1024], F32, tag="mb", name="mt")
                psv = ps[:, :].rearrange("p (b n) -> p b n", b=2)[:, :, 0:N]
                mtv = mt[:, :].rearrange("p (b n) -> p b n", b=2)[:, :, 0:N]
                act(mtv, psv, AF.Exp, scale=SCALE)
                if j == a // 4:
                    for b in range(B):
                        nc.gpsimd.affine_select(
                            out=mt[:, 512 * b + N - 128: 512 * b + N],
                            in_=mt[:, 512 * b + N - 128: 512 * b + N],
                            compare_op=OP.is_ge, fill=0.0, base=0,
                            pattern=[[-1, 128]], channel_multiplier=1)
                dst = arow[:, 512 * j: 512 * j + N]
                nc.vector.tensor_scalar(dst, mt[:, 0:N], sca[(h, 0)][:, a:a + 1],
                                        None, OP.mult)
                nc.vector.scalar_tensor_tensor(dst, mt[:, 512:512 + N],
                                               sca[(h, 1)][:, a:a + 1], dst,
                                               OP.mult, OP.add)
            nc.gpsimd.dma_start(oa[h, 128 * a:128 * (a + 1), 0:width], arow[:, 0:width])

        # attention pass A for both heads, then the collective immediately
        # (pass-B gpsimd work must sit behind the collective in queue order)
        for qi in range(4):
            passA_block(0, qi)
        prep_scales(0)
        for qi in range(4):
            passA_block(1, qi)
        prep_scales(1)

        attn_scope.__exit__(None, None, None)
        osb_cm.__exit__(None, None, None)
        ptp_cm.__exit__(None, None, None)

        nc.gpsimd.collective_compute(
            "AllToAll", OP.bypass, replica_groups=[list(range(NC))],
            ins=[a2a_in.opt()], outs=[a2a_out.opt()])

        for a in range(15, -1, -1):
            passB_chunk(0, a)

        # ---------------- token-sharded proj + LN2; MLP interleaved with head-1 maps ----------------
        with (
            tc.tile_pool(name="xmp", bufs=1) as xmp,
            tc.tile_pool(name="lt2", bufs=1) as lt2,
            tc.tile_pool(name="mst", bufs=4) as mst,
            tc.tile_pool(name="wfq", bufs=1) as wfqp,
            tc.tile_pool(name="gwp", bufs=1) as gwp,
            tc.tile_pool(name="yp", bufs=2) as yp,
        ):
            xm_sb = xmp.tile([128, 4, C], F32)
            r2_col = xmp.tile([128, 4], F32)
            ln2T = lt2.tile([128, 8, TPC], BF16)
            with (
                tc.tile_pool(name="ocp", bufs=1) as ocp,
                tc.tile_pool(name="ln2p", bufs=1) as ln2p,
            ):
                oc_sb = ocp.tile([128, 8, TPC], BF16)
                nc.sync.dma_start(oc_sb[:, :, :], a2a_out.rearrange("a p t -> p a t"))
                wp_sb = ocp.tile([128, 8, C], BF16)
                nc.sync.dma_start(wp_sb[:, :, :], wp.rearrange("(a p) n -> p a n", p=128))
                for t4 in range(4):
                    ps01 = pw.tile([128, 1024], F32, tag="w", name="ps01")
                    for ci in range(8):
                        lh = oc_sb[:, ci, 128 * t4:128 * (t4 + 1)]
                        mm(ps01[:, 0:512], lh, wp_sb[:, ci, 0:512], start=(ci == 0), stop=(ci == 7))
                        mm(ps01[:, 512:1024], lh, wp_sb[:, ci, 512:C], start=(ci == 0), stop=(ci == 7))
                    xsl_t = ln2p.tile([128, C], F32, tag="xs", name="xsl_t")
                    nc.sync.dma_start(xsl_t[:, :], xsl[128 * t4:128 * (t4 + 1), :])
                    nc.vector.tensor_tensor(xm_sb[:, t4, :], ps01[:, :], xsl_t[:, :], OP.add)
                    nc.vector.tensor_tensor(xm_sb[:, t4, :], xm_sb[:, t4, :], bp_bc[:, :], OP.add)

                    # ---- LN2 + transpose to ln2T (bf16) ----
                    bst = mst.tile([128, 2, 6], F32, tag="bst", name="bst")
                    nc.vector.bn_stats(bst[:, 0, :], xm_sb[:, t4, 0:512])
                    nc.vector.bn_stats(bst[:, 1, :], xm_sb[:, t4, 512:C])
                    mv = mst.tile([128, 2], F32, tag="mv", name="mv")
                    nc.vector.bn_aggr(mv[:, :], bst[:, :, :])
                    std2 = mst.tile([128, 1], F32, tag="sd2", name="std2")
                    act(std2[:, :], mv[:, 1:2], AF.Sqrt, bias=eps_col[:, :])
                    nc.vector.reciprocal(r2_col[:, t4:t4 + 1], std2[:, :])
                    ln2_t = ln2p.tile([128, C], F32, tag="ln2", name="ln2_t")
                    nc.vector.scalar_tensor_tensor(
                        ln2_t[:, :], xm_sb[:, t4, :], mv[:, 0:1],
                        r2_col[:, t4:t4 + 1].to_broadcast((128, C)),
                        OP.subtract, OP.mult)
                    for ci in range(8):
                        pst = pw.tile([128, 128], F32, tag="w", name="pst")
                        nc.tensor.transpose(pst[:, :], ln2_t[:, ci * 128:(ci + 1) * 128], ident[:, :])
                        nc.vector.tensor_copy(ln2T[:, ci, 128 * t4:128 * (t4 + 1)], pst[:, :])

            # ---------------- MLP in quarters, head-1 map chunks interleaved ----------------
            wf3 = wf.rearrange("(a p) n -> p a n", p=128)
            mlp_part = xmp.tile([128, 4, C], F32)
            NQ = 4
            MQ = 32 // NQ
            for quar in range(NQ):
                wfq_t = wfqp.tile([128, 8, MQ * 128], BF16, tag="wfq", name="wfq_t")
                nc.sync.dma_start(wfq_t[:, :, :],
                                  wf3[:, :, quar * MQ * 128:(quar + 1) * MQ * 128])
                g_q = gwp.tile([128, MQ, TPC], BF16, tag="g", name="g_q")
                wf2_q = gwp.tile([128, MQ, C], BF16, tag="wf2", name="wf2_q")
                for m in range(MQ):
                    mg = quar * MQ + m
                    ps1 = pw.tile([128, 512], F32, tag="w", name="psf1")
                    for ci in range(8):
                        mm(ps1[:, :], wfq_t[:, ci, m * 128:(m + 1) * 128], ln2T[:, ci, :],
                           start=(ci == 0), stop=(ci == 7))
                    act(g_q[:, m, :], ps1[:, :], AF.Gelu, bias=bf_sb[:, mg:mg + 1])
                    nc.sync.dma_start(wf2_q[:, m, :], wf2[mg * 128:(mg + 1) * 128, :])
                for t4 in range(4):
                    ps2 = pa.tile([128, 1024], F32, tag="a", name="psf2")
                    for m in range(MQ):
                        lh = g_q[:, m, 128 * t4:128 * (t4 + 1)]
                        mm(ps2[:, 0:512], lh, wf2_q[:, m, 0:512], start=(m == 0), stop=(m == MQ - 1))
                        mm(ps2[:, 512:1024], lh, wf2_q[:, m, 512:C], start=(m == 0), stop=(m == MQ - 1))
                    if quar == 0:
                        nc.vector.tensor_copy(mlp_part[:, t4, :], ps2[:, :])
                    elif quar < NQ - 1:
                        nc.vector.tensor_tensor(mlp_part[:, t4, :], ps2[:, :],
                                                mlp_part[:, t4, :], OP.add)
                    else:
                        y_t = yp.tile([128, C], F32, tag="y", name="y_t")
                        nc.vector.tensor_tensor(y_t[:, :], ps2[:, :],
                                                mlp_part[:, t4, :], OP.add)
                        nc.vector.tensor_tensor(y_t[:, :], y_t[:, :], xm_sb[:, t4, :], OP.add)
                        nc.sync.dma_start(oy[128 * t4:128 * (t4 + 1), :], y_t[:, :])
                for a in range(15 - 4 * quar, 11 - 4 * quar, -1):
                    passB_chunk(1, a)

        arp_cm.__exit__(None, None, None)
        mbp_cm.__exit__(None, None, None)
        nlp_cm.__exit__(None, None, None)

# ----------------------------------------------------------------------------
# Host side: runner with cached compile, input prep, output assembly
# ----------------------------------------------------------------------------

_STATE = {}


def _get_runner():
    if "run" in _STATE:
        return _STATE["run"]

    nc = _build_program()
    bass2jax.install_neuronx_cc_hook()

    partition_name = nc.partition_id_tensor.name if nc.partition_id_tensor else None
    in_names, out_names, out_avals = [], [], []
    for alloc in nc.m.functions[0].allocations:
        if not isinstance(alloc, mybir.MemoryLocationSet):
            continue
        name = alloc.memorylocations[0].name
        if alloc.kind == "ExternalInput":
            if name != partition_name:
                in_names.append(name)
        elif alloc.kind == "ExternalOutput":
            shape = tuple(alloc.tensor_shape)
            dtype = mybir.dt.np(alloc.dtype)
            out_names.append(name)
            out_avals.append(jax.core.ShapedArray(shape, dtype))
    n_params = len(in_names)
    full_in_names = in_names + out_names
    if partition_name is not None:
        full_in_names.append(partition_name)

    donate = tuple(range(n_params, n_params + len(out_names)))

    def _body(*args):
        operands = list(args)
        if partition_name is not None:
            operands.append(bass2jax.partition_id_tensor())
        outs = bass2jax._bass_exec_p.bind(
            *operands,
            out_avals=tuple(out_avals),
            in_names=tuple(full_in_names),
            out_names=tuple(out_names),
            lowering_input_output_aliases=(),
            sim_require_finite=True,
            sim_require_nnan=True,
            nc=nc,
        )
        return tuple(outs)

    devices = jax.devices()[:NC]
    mesh = Mesh(np.asarray(devices), ("core",))
    n_in = n_params + len(out_names)
    sharded = jax.jit(
        _shard_map(_body, mesh=mesh,
                   in_specs=(PartitionSpec("core"),) * n_in,
                   out_specs=(PartitionSpec("core"),) * len(out_names),
                   check_rep=False),
        donate_argnums=donate, keep_unused=True)

    sharding = NamedSharding(mesh, PartitionSpec("core"))
    zero_shapes = [(NC * av.shape[0], *av.shape[1:]) for av in out_avals]
    zero_dtypes = [av.dtype for av in out_avals]

    @jax.jit
    def _mk_zeros():
        return tuple(jnp.zeros(s, d) for s, d in zip(zero_shapes, zero_dtypes))

    def make_zeros():
        return jax.device_put(_mk_zeros(), tuple([sharding] * len(out_avals)))

    def put_inputs(in_maps):
        arrs = []
        for name in in_names:
            cat = np.concatenate([np.asarray(m[name]) for m in in_maps], axis=0)
            arrs.append(jax.device_put(cat, sharding))
        return arrs

    runner = {
        "nc": nc, "sharded": sharded, "in_names": in_names, "out_names": out_names,
        "out_avals": out_avals, "make_zeros": make_zeros, "put_inputs": put_inputs,
        "mesh": mesh,
    }
    _STATE["run"] = runner
    return runner


def _round_f32r(a):
    """Round fp32 to the PE's fp32r format (11-bit mantissa, RNE-ish)."""
    b = np.ascontiguousarray(a, np.float32).view(np.uint32)
    b = ((b + np.uint32(0x800)) & np.uint32(0xFFFFF000)).astype(np.uint32)
    return b.view(np.float32)


def _prep_inputs(inputs):
    x = np.asarray(inputs["x"], np.float32)
    ln1_w = np.asarray(inputs["ln1_w"], np.float32)
    ln1_b = np.asarray(inputs["ln1_b"], np.float32)
    w_qkv = np.asarray(inputs["w_qkv"], np.float32)
    b_qkv = np.asarray(inputs["b_qkv"], np.float32)
    w_proj = np.asarray(inputs["w_proj"], np.float32)
    b_proj = np.asarray(inputs["b_proj"], np.float32)
    ln2_w = np.asarray(inputs["ln2_w"], np.float32)
    ln2_b = np.asarray(inputs["ln2_b"], np.float32)
    w_fc = np.asarray(inputs["w_fc"], np.float32)
    b_fc = np.asarray(inputs["b_fc"], np.float32)
    w_fc2 = np.asarray(inputs["w_fc2"], np.float32)
    b_fc2 = np.asarray(inputs["b_fc2"], np.float32)

    x2d = np.ascontiguousarray(x.reshape(BT, C))
    xt_r = None  # set after r is computed

    # LN1 per-token inverse std (cheap O(BT*C) host work)
    s = x2d.sum(axis=1, dtype=np.float32)
    ss = np.einsum("tc,tc->t", x2d, x2d, dtype=np.float32)
    var = ss / C - (s / C) ** 2
    r = (1.0 / np.sqrt(var + EPS)).astype(np.float32)
    xt_r = _round_f32r(x2d.T * r[None, :])

    Wd = ln1_w[:, None] * w_qkv
    Wc = np.ascontiguousarray(Wd - Wd.mean(axis=0, keepdims=True))
    bias_qkv = ln1_b @ w_qkv + b_qkv

    Wdfc = (ln2_w[:, None] * w_fc).astype(ml_dtypes.bfloat16)
    bias_fc = (ln2_b @ w_fc + b_fc).astype(np.float32)
    wf2_bf = w_fc2.astype(ml_dtypes.bfloat16)

    in_maps = []
    for c in range(NC):
        qcols = slice(128 * c, 128 * (c + 1))
        kcols = slice(C + 128 * c, C + 128 * (c + 1))
        vcols = slice(2 * C + 128 * c, 2 * C + 128 * (c + 1))
        in_maps.append({
            "xt": xt_r,
            "xsl": np.ascontiguousarray(x2d[TPC * c:TPC * (c + 1)]),
            "wq": _round_f32r(Wc[:, qcols]),
            "wk": _round_f32r(Wc[:, kcols]),
            "wv": _round_f32r(Wc[:, vcols]),
            "bq": np.ascontiguousarray(bias_qkv[qcols].reshape(128, 1)),
            "bk": np.ascontiguousarray(bias_qkv[kcols].reshape(128, 1)),
            "bv": np.ascontiguousarray(bias_qkv[vcols].reshape(128, 1)),
            "wp": w_proj.astype(ml_dtypes.bfloat16),
            "bp": np.ascontiguousarray(b_proj.reshape(1, C)),
            "wf": Wdfc,
            "bf": np.ascontiguousarray(bias_fc.reshape(32, 128).T),
            "wf2": wf2_bf,
        })
    return in_maps, b_fc2


def _assemble(per_core, b_fc2):
    x_out = np.concatenate([per_core[c]["oy"] for c in range(NC)], axis=0)
    x_out = (x_out + b_fc2[None, :]).astype(np.float32).reshape(B, T, C)
    attn = np.concatenate([per_core[c]["oa"] for c in range(NC)], axis=0)
    return x_out, attn


def run_on_device(in_maps):
    """Execute one step; returns per-core dict of outputs (host numpy)."""
    r = _get_runner()
    in_dev = r["put_inputs"](in_maps)
    zeros = r["make_zeros"]()
    outs = r["sharded"](*in_dev, *zeros)
    per_core = []
    for c in range(NC):
        d = {}
        for i, name in enumerate(r["out_names"]):
            av = r["out_avals"][i]
            d[name] = np.asarray(outs[i]).reshape(NC, *av.shape)[c]
        per_core.append(d)
    return per_core


def kernel(**inputs):
    in_maps, b_fc2 = _prep_inputs(inputs)
    per_core = run_on_device(in_maps)
    return _assemble(per_core, b_fc2)


if __name__ == "__main__":
    nc = _build_program()
    print("build ok:", len(nc.m.functions[0].allocations), "allocations")
